# revision 28
# baseline (speedup 1.0000x reference)
"""DivergentAttention Trainium2 kernel (8 NeuronCores, Bass/Tile).

Problem: GPT-2 style causal self-attention (B=2, S=2048, D=1024, H=16,
hd=64) where heads 0/1/2 re-weight their attention toward a token region
(first/middle/last third of the sequence) with factor 1.6 and renormalize.

Key identity: softmax(s)*m / sum(softmax(s)*m) == softmax(s + log m), so the
per-head region reweight folds into an additive per-(head, key-position)
bias on the scores -- no second normalization pass needed. Scores are small
(|s|<~5) so the max-subtraction pass is skipped entirely.

Sharding: tensor-parallel over (batch, head-group): core c handles batch
c//4 and heads [4*(c%4), 4*(c%4)+4). Each core computes the QKV projection
for its 4 heads, full causal attention, and its partial c_proj; the host
sums the 8 partials and adds c_proj_b.

Layouts (all transposed so no on-chip transposes are ever needed):
  - hiddenT  [D, S]  (host-transposed)  -> QKV matmuls contract over D;
    the contraction (ko) loop is OUTER with 8 resident PSUM groups so PE
    starts as soon as the first 128-row chunk of hiddenT/w lands.
  - qkT      [4*128, S]: q(h0,h1) | q(h2,h3) | k(h0,h1) | k(h2,h3); head at
    partition offset 64*(h%2) within its 128-tile.
  - scoresT  [sk-tile=128, sq] = kT.T @ qT; causal => only sq >= 128*t is
    computed; the diagonal 128x128 block gets a 0/1 triangular mask
    multiply AFTER the exp (on GPSIMD, all-SBUF, so it never stalls the
    ScalarE exp stream -- exp(-inf)=0 is replaced by exp(s)*0).
  - exp via ScalarE with scale=1/8 and per-partition bias log(mult[h, sk]).
  - v        [S, hd] natural ([128, 16, 4, 65] with a ones column at index
    64) so out.T = v_aug.T @ attnT gives both out.T (rows 0..63) and the
    softmax denominator (row 64) in one accumulation.
  - denominators: copy [65,1024] PSUM->SBUF (frees the accumulation bank
    early), DVE reciprocal, one DRAM bounce + partition-broadcast DMA per
    head (on the gpsimd queue, off the bulk-DMA queue), DVE multiply into
    ao2 [128, 2, S] float32r with heads partition-interleaved (h even ->
    partitions 0..63, h odd -> 64..127).
  - c_proj: two K=128 matmuls per output tile against pw2 [128, 2, D]
    (head-pair rows packed to match ao2).

All matmuls run in float32r (TF32-like, full PE rate at N>=256; measured
~1.5e-4 relative error at K=1024).
"""

import numpy as np

import concourse.bass as bass
import concourse.tile as tile
from concourse import mybir
from concourse import bass_utils, bass2jax

# ---------------------------------------------------------------- constants
B, S, D, H, HD = 2, 2048, 1024, 16, 64
NCORES = 8
HPC = 4              # heads per core
GROUPS = 4           # head groups
FOCUS = 1.6
HEAD_REGION = {0: 0, 1: 1, 2: 2}
DT_R = mybir.dt.float32r
DT_F = mybir.dt.float32

# ------------------------------------------------- walrus multi-wait fixup
# This container's walrus accepts only ONE sync-wait per TPB instruction,
# but Tile attaches one wait per dependency proc. Rewrite the BIR JSON just
# before walrus: hoist all-but-one wait of a multi-wait instruction onto
# standalone same-engine NoOps inserted immediately before it (same-engine
# program order is preserved, so semantics are unchanged).
try:
    import orjson as _json
except ImportError:  # pragma: no cover
    import json as _json

_orig_compile_bir_kernel = bass_utils.compile_bir_kernel
_wfix_counter = [0]


def _fix_bir(bir_json):
    d = _json.loads(bir_json)
    changed = False
    for fn in d.get("functions", []):
        for blk in fn.get("blocks", []):
            out = []
            for inst in blk.get("instructions", []):
                si = inst.get("sync_info")
                if si:
                    waits = si.get("on_wait") or []
                    if len(waits) > 1:
                        changed = True
                        for w in waits[:-1]:
                            _wfix_counter[0] += 1
                            nop = {
                                "engine": inst["engine"],
                                "ins": [],
                                "name": f"I-wfix-{_wfix_counter[0]}",
                                "opcode": "NoOp",
                                "outs": [],
                                "sync_info": {"on_update": [], "on_wait": [w]},
                            }
                            if "debug" in inst:
                                nop["debug"] = inst["debug"]
                            out.append(nop)
                        si["on_wait"] = waits[-1:]
                out.append(inst)
            blk["instructions"] = out
    return _json.dumps(d) if changed else bir_json


def _patched_compile_bir_kernel(bir_json, tmpdir, neff_name="file.neff"):
    return _orig_compile_bir_kernel(_fix_bir(bir_json), tmpdir, neff_name=neff_name)


def _install_waitfix():
    bass_utils.compile_bir_kernel = _patched_compile_bir_kernel
    bass2jax.compile_bir_kernel = _patched_compile_bir_kernel


_install_waitfix()

# ---------------------------------------------------------------- program


def build_program():
    """One SPMD Bass program; per-core differences come in via inputs."""
    nc = bass.Bass()
    NT = S // 128       # 16 sk tiles
    KO = D // 128       # 8 contraction chunks

    hiddenT = nc.dram_tensor("hiddenT", [D, S], DT_R, kind="ExternalInput")
    w_qkv = nc.dram_tensor("w_qkv", [D, 768], DT_R, kind="ExternalInput")
    bqk = nc.dram_tensor("bqk", [128, 4], DT_F, kind="ExternalInput")
    bv_rep = nc.dram_tensor("bv_rep", [128, 256], DT_F, kind="ExternalInput")
    projw = nc.dram_tensor("projw", [128, 2, D], DT_R, kind="ExternalInput")
    diag_mask = nc.dram_tensor("diag_mask", [128, 128], DT_R, kind="ExternalInput")
    logmult = nc.dram_tensor("logmult", [128, HPC, NT], DT_F, kind="ExternalInput")
    out = nc.dram_tensor("out", [S, D], DT_F, kind="ExternalOutput")

    with tile.TileContext(nc) as tc:
        with tc.tile_pool(name="persist", bufs=1) as persist, \
             tc.tile_pool(name="dram", bufs=4, space="DRAM") as dram:

            # ---- persistent SBUF ----
            qk_sb = persist.tile([128, 4, S], DT_R)        # 4 MB
            v_sb = persist.tile([128, NT, HPC, 65], DT_R)  # ~2.1 MB
            ao2 = persist.tile([128, 2, S], DT_R)          # attn_outT, 2 MB
            bqk_sb = persist.tile([128, 4], DT_F)
            bv_sb = persist.tile([128, 256], DT_F)
            pw_sb = persist.tile([128, 2, D], DT_R)        # 1 MB
            dm_sb = persist.tile([128, 128], DT_R)
            lm_sb = persist.tile([128, HPC, NT], DT_F)

            nc.sync.dma_start(bqk_sb, bqk[:, :])
            nc.vector.memset(v_sb[:, :, :, 64:65].bitcast(DT_F), 1.0)

            # ================= phase 1: QKV projection =================
            # ko (contraction) outer, 8 resident PSUM groups: PE consumes
            # each 1.4 MB (hiddenT+w) chunk as it arrives from HBM.
            with tc.tile_pool(name="p1sb", bufs=1) as p1sb, \
                 tc.tile_pool(name="p1ps", bufs=8, space="PSUM") as p1ps:
                hT = p1sb.tile([128, KO, S], DT_R)        # 8 MB
                w_sb = p1sb.tile([128, KO, 768], DT_R)    # 3 MB
                hT_src = hiddenT.rearrange("(ko p) s -> p ko s", p=128)
                w_src = w_qkv.rearrange("(ko p) n -> p ko n", p=128)
                # split the 11 MB load across the two HWDGE queues (SP + ACT)
                for ko in range(KO):
                    q = nc.sync if ko % 2 == 0 else nc.scalar
                    qo = nc.scalar if ko % 2 == 0 else nc.sync
                    q.dma_start(hT[:, ko, :], hT_src[:, ko, :])
                    qo.dma_start(w_sb[:, ko, :], w_src[:, ko, :])
                    if ko == 0:
                        # small tensors ride behind the first bulk chunk;
                        # all are needed only later (v copies / attention /
                        # c_proj)
                        nc.sync.dma_start(bv_sb, bv_rep[:, :])
                        nc.scalar.dma_start(dm_sb, diag_mask[:, :])
                        nc.scalar.dma_start(lm_sb, logmult[:, :, :])
                        nc.scalar.dma_start(pw_sb, projw[:, :, :])

                # qT/kT: out[n-tile, s] = w.T @ hiddenT, two rounds of 8 psums
                for rnd in range(2):
                    ps8 = [p1ps.tile([128, 512], DT_F, tag="g", name=f"q{rnd}{i}")
                           for i in range(8)]
                    for ko in range(KO):
                        for i in range(8):
                            nt, sc = 2 * rnd + i // 4, i % 4
                            nc.tensor.matmul(
                                ps8[i],
                                w_sb[:, ko, 128 * nt:128 * nt + 128],
                                hT[:, ko, 512 * sc:512 * sc + 512],
                                start=(ko == 0), stop=(ko == KO - 1),
                            )
                    for i in range(8):
                        nt, sc = 2 * rnd + i // 4, i % 4
                        nc.scalar.activation(
                            qk_sb[:, nt, 512 * sc:512 * sc + 512], ps8[i],
                            mybir.ActivationFunctionType.Identity,
                            bias=bqk_sb[:, nt:nt + 1], scale=1.0,
                        )

                # v natural: out[s-tile, (h,hd)] = hidden @ wv
                for rnd in range(2):
                    ps8 = [p1ps.tile([128, 512], DT_F, tag="g", name=f"v{rnd}{i}")
                           for i in range(8)]
                    for ko in range(KO):
                        for i in range(8):
                            st = 8 * rnd + i
                            nc.tensor.matmul(
                                ps8[i][:, 0:256],
                                hT[:, ko, 128 * st:128 * st + 128],
                                w_sb[:, ko, 512:768],
                                start=(ko == 0), stop=(ko == KO - 1),
                            )
                    for i in range(8):
                        st = 8 * rnd + i
                        nc.vector.tensor_add(
                            out=v_sb[:, st, :, 0:64],
                            in0=ps8[i][:, 0:256].rearrange("p (h d) -> p h d", d=64),
                            in1=bv_sb.rearrange("p (h d) -> p h d", d=64),
                        )

            # ================= phase 2: attention per head =================
            with tc.tile_pool(name="p2sb", bufs=6) as p2sb, \
                 tc.tile_pool(name="p2cp", bufs=6) as p2cp, \
                 tc.tile_pool(name="p2rep", bufs=4) as p2rep, \
                 tc.tile_pool(name="p2row", bufs=4) as p2row, \
                 tc.tile_pool(name="p2sc", bufs=2, space="PSUM") as p2sc, \
                 tc.tile_pool(name="p2av", bufs=4, space="PSUM") as p2av:
                for lh in range(HPC):
                    bp = 64 * (lh % 2)
                    q_nt = lh // 2
                    k_nt = 2 + lh // 2
                    av_ps = [p2av.tile([128, 512], DT_F, tag="av", name=f"av{c}")
                             for c in range(4)]
                    def drain_chunk(c):
                        # av_ps[c] fully accumulated: copy to SBUF (frees the
                        # PSUM bank for the next head), reciprocal of the
                        # denominator row, DRAM-bounce partition-broadcast,
                        # then normalize into ao2 (GPSIMD: all-SBUF operands,
                        # keeps DVE off the critical path).
                        cp = p2cp.tile([65, 512], DT_F, tag="avcp", name=f"cp{c}")
                        nc.vector.tensor_copy(cp, av_ps[c][0:65, :])
                        rec = p2row.tile([1, 512], DT_F, tag="rec")
                        nc.vector.reciprocal(rec, cp[64:65, :])
                        dtile = dram.tile([1, 512], DT_F)
                        nc.gpsimd.dma_start(dtile, rec)
                        rep = p2rep.tile([64, 512], DT_F, tag="rep")
                        src = dtile[0, :]
                        bcast = bass.AP(
                            tensor=src.tensor, offset=src.offset,
                            ap=[[0, 64]] + [list(pr) for pr in src.ap],
                        )
                        nc.gpsimd.dma_start(rep, bcast)
                        nc.gpsimd.tensor_mul(
                            out=ao2[bp:bp + 64, lh // 2,
                                    512 * c:512 * (c + 1)],
                            in0=cp[0:64, :],
                            in1=rep,
                        )

                    for t in range(NT):
                        lhsT_k = qk_sb[bp:bp + 64, k_nt, 128 * t:128 * t + 128]
                        v_aug = v_sb[:, t, lh, :]
                        if t in (4, 8, 12):
                            drain_chunk(t // 4 - 1)  # chunk done at t-1
                        for p in range(t // 8, 2):
                            gs = max(1024 * p, 128 * t)       # global col start
                            ge = 1024 * (p + 1)
                            width = ge - gs
                            sc_ps = p2sc.tile([128, 1024], DT_F, tag="sc")
                            # scores (single K=64 matmul per <=512 piece)
                            off = 0
                            while off < width:
                                w512 = min(512, width - off)
                                nc.tensor.matmul(
                                    sc_ps[:, off:off + w512],
                                    lhsT_k,
                                    qk_sb[bp:bp + 64, q_nt, gs + off:gs + off + w512],
                                    start=True, stop=True,
                                )
                                off += w512
                            # exp (scale 1/sqrt(hd)=1/8, bias log-mult)
                            at_sb = p2sb.tile([128, 1024], DT_R, tag="attnT")
                            nc.scalar.activation(
                                at_sb[:, :width], sc_ps[:, :width],
                                mybir.ActivationFunctionType.Exp,
                                bias=lm_sb[:, lh, t:t + 1], scale=0.125,
                            )
                            # causal 0/1 mask on the diagonal block, after the
                            # exp (GPSIMD, all-SBUF: never gates the ACT
                            # stream, only the following av matmul)
                            if gs == 128 * t:
                                nc.gpsimd.tensor_mul(
                                    out=at_sb[:, 0:128], in0=at_sb[:, 0:128],
                                    in1=dm_sb,
                                )
                            # accumulate out.T (rows 0-63) + denom (row 64)
                            off = 0
                            while off < width:
                                g0 = gs + off
                                c = g0 // 512
                                w512 = min(512, 512 * (c + 1) - g0)
                                t_last = min(NT - 1, 4 * c + 3)
                                nc.tensor.matmul(
                                    av_ps[c][0:65, (g0 % 512):(g0 % 512) + w512],
                                    v_aug,
                                    at_sb[:, off:off + w512],
                                    start=(t == 0), stop=(t == t_last),
                                )
                                off += w512
                    drain_chunk(3)

            # ================= phase 3: c_proj partial =================
            with tc.tile_pool(name="p3sb", bufs=6) as p3sb, \
                 tc.tile_pool(name="p3ps", bufs=4, space="PSUM") as p3ps:
                for st in range(NT):
                    for ec in range(2):
                        ps = p3ps.tile([128, 512], DT_F, tag="pr")
                        for j in range(2):
                            nc.tensor.matmul(
                                ps,
                                ao2[:, j, 128 * st:128 * st + 128],
                                pw_sb[:, j, 512 * ec:512 * ec + 512],
                                start=(j == 0), stop=(j == 1),
                            )
                        o_sb = p3sb.tile([128, 512], DT_F, tag="out")
                        k = 2 * st + ec
                        if k % 3 == 0:
                            nc.scalar.copy(o_sb, ps)
                        else:
                            nc.vector.tensor_copy(o_sb, ps)
                        oq = (nc.scalar, nc.sync, nc.sync)[k % 3]
                        oq.dma_start(
                            out[128 * st:128 * st + 128, 512 * ec:512 * ec + 512],
                            o_sb,
                        )
    return nc


_NC = None


def _get_nc():
    global _NC
    if _NC is None:
        _NC = build_program()
    return _NC


# ---------------------------------------------------------------- host prep

def make_in_maps(hidden_states, c_attn_w, c_attn_b, c_proj_w):
    first_end = S // 3
    second_end = 2 * S // 3
    pos = np.arange(S)
    regions = [pos < first_end,
               (pos >= first_end) & (pos < second_end),
               pos >= second_end]
    mult = np.ones((H, S), dtype=np.float64)
    for h, r in HEAD_REGION.items():
        mult[h] = 1.0 + (FOCUS - 1.0) * regions[r].astype(np.float64)
    logm = np.log(mult).astype(np.float32)  # [H, S]

    p = np.arange(128)[:, None]
    j = np.arange(128)[None, :]
    diag = (j >= p).astype(np.float32)  # 0/1 keep-mask, applied post-exp

    in_maps = []
    for c in range(NCORES):
        b, g = divmod(c, GROUPS)
        h0 = HPC * g
        cs = slice(256 * g, 256 * g + 256)
        w_qkv = np.concatenate(
            [c_attn_w[:, cs], c_attn_w[:, 1024:2048][:, cs],
             c_attn_w[:, 2048:3072][:, cs]], axis=1,
        ).astype(np.float32)
        bqk = np.concatenate(
            [c_attn_b[cs], c_attn_b[1024:2048][cs]]
        ).reshape(4, 128).T.copy().astype(np.float32)
        bv = np.broadcast_to(
            c_attn_b[2048:3072][cs], (128, 256)
        ).astype(np.float32).copy()
        # pw2[p, j, e]: head pair j=(2j, 2j+1); p<64 -> head 2j row p,
        # p>=64 -> head 2j+1 row p-64  (matches ao2 partition interleave)
        pw = c_proj_w[64 * h0:64 * h0 + 256, :].reshape(2, 128, D)
        pw = np.ascontiguousarray(pw.transpose(1, 0, 2)).astype(np.float32)
        lm = logm[h0:h0 + HPC].reshape(HPC, S // 128, 128)
        lm = np.ascontiguousarray(lm.transpose(2, 0, 1)).astype(np.float32)
        in_maps.append({
            "hiddenT": np.ascontiguousarray(hidden_states[b].T).astype(np.float32),
            "w_qkv": w_qkv,
            "bqk": bqk,
            "bv_rep": bv,
            "projw": pw,
            "diag_mask": diag,
            "logmult": lm,
        })
    return in_maps


def run_cores(in_maps, trace=False, **kw):
    from concourse.bass_utils import run_bass_kernel_spmd
    nc = _get_nc()
    return run_bass_kernel_spmd(nc, in_maps, core_ids=list(range(NCORES)),
                                trace=trace, **kw)


def kernel(hidden_states, c_attn_w, c_attn_b, c_proj_w, c_proj_b):
    hidden_states = np.asarray(hidden_states, dtype=np.float32)
    c_attn_w = np.asarray(c_attn_w, dtype=np.float32)
    c_attn_b = np.asarray(c_attn_b, dtype=np.float32)
    c_proj_w = np.asarray(c_proj_w, dtype=np.float32)
    c_proj_b = np.asarray(c_proj_b, dtype=np.float32)

    in_maps = make_in_maps(hidden_states, c_attn_w, c_attn_b, c_proj_w)
    res = run_cores(in_maps)
    out = np.zeros((B, S, D), dtype=np.float32)
    for c in range(NCORES):
        out[c // GROUPS] += res.results[c]["out"]
    out += c_proj_b[None, None, :]
    return out
